# revision 39
# baseline (speedup 1.0000x reference)
"""GatedLSTM Trainium2 kernel: 8-core hidden-sharded recurrence.

Strategy
--------
B=64, S=512, I=H=1024. Gates f, o, c (input gate `i` is unused by the
reference). Work is split across the 8 NeuronCores by the *hidden* dim:
core c owns rows [128c, 128c+128) of all three gates. This keeps every
per-step matmul at full PE width (stationary U tiles are 128x128) instead
of the batch-parallel split, which would leave the PE array 8/128 occupied
and re-stream all of U through every core each step.

Phase 1 (parallel): X_g = x @ W_g.T + b_g for the core's 128-row slice,
computed as 8 K-tile matmuls per 512-column chunk from a pre-transposed
bf16 copy of x, written to DRAM scratch in the step-major layout phase 2
consumes.

Phase 2 (recurrent, 512 steps): each step does 24 bf16 matmuls
(3 gates x 8 K-tiles) of U-tile[128,128] @ hT[128,64] accumulating in
PSUM, adds the X slice (DVE), applies sigmoid/tanh (ACT), updates c and h
(DVE), then broadcasts its 16KB hT slice to the other 7 cores through
`remote_dma_broadcast` (SWDGE P2P SBUF->SBUF writes with remote semaphore
increments). Waits: each peer's broadcast bumps the receiver's parity
semaphore by 2, so a step waits for +14 since the previous same-parity
exchange. The h state buffer is double buffered by step parity.

Cross-core slot mapping: relative XOR routing lands sender s in slot k of
receiver r with tpb(s) = tpb(r) XOR k. The device-order -> tpb mapping of
this container was probed; M_MAP[c][k] below is "which device's h slice
lives in slot k on device c". Per-core U inputs are permuted accordingly
on the host. A stamp exchange validates the map at runtime; on mismatch
the kernel rebuilds the permutation from the observed stamps and reruns.
"""

import numpy as np
import ml_dtypes

import concourse.bass as bass
import concourse.bacc as bacc
import concourse.tile as tile
import concourse.mybir as mybir
from concourse import bass_utils, library_config
from concourse.bass import ds, ts
from concourse.tile_rust import add_dep_helper

B, S, I, H = 64, 512, 1024, 1024
P = 128
NCORES = 8
HS = H // NCORES          # 128 hidden rows per core
KT = H // P               # 8 contraction K-tiles
G = 3                     # gates packed [f | o | c]
CH = 512                  # phase-1 chunk: 8 steps x 64 batch columns
AF = mybir.ActivationFunctionType

# slot k on device c holds the h slice owned by device M_MAP[c][k]
# (probed on this container: devices 4..7 sit on tpbs 6,7,4,5).
M_MAP = [
    [0, 1, 2, 3, 6, 7, 4, 5],
    [1, 0, 3, 2, 7, 6, 5, 4],
    [2, 3, 0, 1, 4, 5, 6, 7],
    [3, 2, 1, 0, 5, 4, 7, 6],
    [4, 5, 6, 7, 2, 3, 0, 1],
    [5, 4, 7, 6, 3, 2, 1, 0],
    [6, 7, 4, 5, 0, 1, 2, 3],
    [7, 6, 5, 4, 1, 0, 3, 2],
]

BF16 = ml_dtypes.bfloat16


def build_program(s_steps: int = S, debug: bool = False):
    assert s_steps % 8 == 0
    dt = mybir.dt
    nc = bacc.Bacc("TRN2", target_bir_lowering=False, debug=False,
                   num_devices=NCORES)

    xT = nc.dram_tensor("xT", [KT, P, s_steps * B], dt.bfloat16,
                        kind="ExternalInput")
    # weight tensors are staged in SBUF-layout: [partition, gate, ktile, col]
    wT = nc.dram_tensor("wT", [P, G, KT, P], dt.bfloat16, kind="ExternalInput")
    uT = nc.dram_tensor("uT", [P, G, KT, P], dt.bfloat16, kind="ExternalInput")
    bia = nc.dram_tensor("bias", [P, G], dt.float32, kind="ExternalInput")
    idv = nc.dram_tensor("idv", [P, 8], dt.float32, kind="ExternalInput")
    idn = nc.dram_tensor("ident", [P, P], dt.float32, kind="ExternalInput")

    y = nc.dram_tensor("y", [B, s_steps, HS], dt.float32, kind="ExternalOutput")
    hc = nc.dram_tensor("hc", [2, B, HS], dt.float32, kind="ExternalOutput")
    stamp_o = nc.dram_tensor("stamp", [P, NCORES, 8], dt.float32,
                             kind="ExternalOutput")

    Xpre = nc.dram_tensor("Xpre", [P, s_steps, G * B], dt.float32,
                          kind="Internal")
    if debug:
        dbg_x = nc.dram_tensor("dbg_x", [P, G * B], dt.float32,
                               kind="ExternalOutput")
        dbg_pre = nc.dram_tensor("dbg_pre", [P, G * B], dt.float32,
                                 kind="ExternalOutput")
        dbg_h = nc.dram_tensor("dbg_h", [P, NCORES, B], dt.float32,
                               kind="ExternalOutput")

    # waits on semaphores that only remote cores increment are attached
    # *after* Tile scheduling: the scheduler's single-core sim cannot model
    # remote increments and would report a false deadlock.
    post_waits = []

    with tile.TileContext(nc) as tc:
        u_sb = nc.alloc_sbuf_tensor("u_sb", [P, G, KT, P], dt.bfloat16).ap()
        w_sb = nc.alloc_sbuf_tensor("w_sb", [P, G, KT, P], dt.bfloat16).ap()
        b_sb = nc.alloc_sbuf_tensor("b_sb", [P, G], dt.float32).ap()
        hT = nc.alloc_sbuf_tensor("hT", [P, 2, NCORES, B], dt.bfloat16).ap()
        cT = nc.alloc_sbuf_tensor("cT", [P, B], dt.float32).ap()
        stp = nc.alloc_sbuf_tensor("stp", [P, NCORES, 8], dt.float32).ap()
        ident = nc.alloc_sbuf_tensor("identity", [P, P], dt.float32).ap()

        rsem = [nc.alloc_semaphore("rsem0"), nc.alloc_semaphore("rsem1")]
        ssem = nc.alloc_semaphore("ssem")
        lsem = [nc.alloc_semaphore(f"lsem{i}") for i in range(4)]
        lsem_s = nc.alloc_semaphore("lsem_s")
        tsem = nc.alloc_semaphore("tsem")   # orders preps after prior trigger
        xsem = nc.alloc_semaphore("xsem")   # phase1 -> phase2 Xpre handoff

        nc.gpsimd.load_library(library_config.remote_dma)

        nc.sync.dma_start(u_sb, uT.ap())
        nc.sync.dma_start(w_sb, wT.ap())
        nc.sync.dma_start(b_sb, bia.ap())
        nc.sync.dma_start(ident, idn.ap())
        nc.sync.dma_start(stp[:, 0, :], idv.ap())
        nc.vector.memset(hT[:, 0], 0.0)
        nc.vector.memset(cT, 0.0)

        # stamp exchange: validates the device->slot ownership map
        for k in range(1, NCORES):
            rd = [None] * 8
            rd[k] = (0, k)
            nc.gpsimd.remote_dma_broadcast(stp[:, k, :], stp[:, 0, :],
                                           ssem, lsem_s, rdests=rd)
        prev_trig = nc.gpsimd.trigger_dma(count=None).then_inc(tsem)

        # ---------------- phase 1: X = x @ W.T + b ----------------
        nch = s_steps * B // CH
        with tc.tile_pool(name="xc", bufs=3) as xpool, \
             tc.tile_pool(name="p1ps", bufs=3, space="PSUM") as p1ps, \
             tc.tile_pool(name="p1o", bufs=3) as p1o:
            for ci in range(nch):
                xt = xpool.tile([P, KT, CH], dt.bfloat16)
                for j in range(KT):
                    nc.sync.dma_start(xt[:, j], xT.ap()[j, :, ts(ci, CH)])
                x_outs = []
                for g in range(G):
                    ps = p1ps.tile([P, CH], dt.float32)
                    for j in range(KT):
                        nc.tensor.matmul(ps[:], w_sb[:, g, j], xt[:, j],
                                         start=(j == 0), stop=(j == KT - 1))
                    xo = p1o.tile([P, 8, B], dt.float32)
                    nc.scalar.activation(xo[:], ps[:].rearrange("p (t b) -> p t b", b=B),
                                         AF.Identity, bias=b_sb[:, ds(g, 1)])
                    x_outs.append(nc.sync.dma_start(
                        Xpre.ap()[:, ts(ci, 8), ts(g, B)], xo[:]))
                dnop = nc.vector.nop()
                for o in x_outs:
                    add_dep_helper(dnop.ins, o.ins, sync=True,
                                   reason="Xpre chunk stores complete")
                dnop.then_inc(xsem)

        # ---------------- phase 2: recurrence ----------------
        with tc.tile_pool(name="xq", bufs=8) as xq, \
             tc.tile_pool(name="p2ps", bufs=2, space="PSUM") as p2ps, \
             tc.tile_pool(name="gact", bufs=3) as gact, \
             tc.tile_pool(name="yps", bufs=2, space="PSUM") as yps, \
             tc.tile_pool(name="ysb", bufs=4) as ysb:
            for t in range(s_steps):
                par = t % 2
                npar = 1 - par
                xt_t = xq.tile([P, G * B], dt.float32)
                xnop = nc.vector.nop()
                xnop._wait_ge(xsem, t // 8 + 1)
                xload = nc.sync.dma_start(xt_t[:], Xpre.ap()[:, t, :])
                add_dep_helper(xload.ins, xnop.ins, sync=True,
                               reason="X load after phase-1 chunk")

                ps_fo = p2ps.tile([P, 2 * B], dt.float32)
                ps_cc = p2ps.tile([P, B], dt.float32)
                wait_val = 14 * ((t + 1) // 2)
                # The receive-wait rides on a PE nop anchored after a k=0
                # matmul (k=0 reads the locally written slot 0, which RAW-
                # chains it behind the whole previous step) and before every
                # gate's k=1 matmul (the first reader of remote slots).
                rnop = None
                for g in (2, 0, 1):   # c first so the tanh path starts early
                    dst = ps_cc[:] if g == 2 else ps_fo[:, ts(g, B)]
                    for k in range(NCORES):
                        mm = nc.tensor.matmul(dst, u_sb[:, g, k],
                                              hT[:, par, k, :],
                                              start=(k == 0),
                                              stop=(k == NCORES - 1))
                        if t > 0:
                            if k == 0 and g == 2:
                                rnop = nc.tensor.nop()
                                add_dep_helper(rnop.ins, mm.ins, sync=False,
                                               reason="rwait after local k0")
                                post_waits.append((rnop, rsem[par], wait_val))
                            elif k == 1:
                                add_dep_helper(mm.ins, rnop.ins, sync=False,
                                               reason="remote slots gated")

                pre_cc = gact.tile([P, B], dt.float32)
                nc.vector.tensor_add(pre_cc[:], ps_cc[:], xt_t[:, ds(2 * B, B)])
                g_t = gact.tile([P, B], dt.float32)
                nc.scalar.activation(g_t[:], pre_cc[:], AF.Tanh)

                pre_fo = gact.tile([P, 2 * B], dt.float32)
                nc.vector.tensor_add(pre_fo[:], ps_fo[:], xt_t[:, 0:2 * B])
                fo = gact.tile([P, 2 * B], dt.float32)
                nc.scalar.activation(fo[:], pre_fo[:], AF.Sigmoid)

                if debug and t == 1:
                    nc.sync.dma_start(dbg_x.ap(), xt_t[:])
                    dpre = gact.tile([P, G * B], dt.float32)
                    nc.vector.tensor_copy(dpre[:, 0:2 * B], pre_fo[:])
                    nc.vector.tensor_copy(dpre[:, ds(2 * B, B)], pre_cc[:])
                    nc.sync.dma_start(dbg_pre.ap(), dpre[:])
                    dh = gact.tile([P, NCORES, B], dt.float32)
                    dcp = nc.vector.tensor_copy(dh[:], hT[:, par])
                    add_dep_helper(dcp.ins, rnop.ins, sync=True,
                                   reason="dbg after receive wait")
                    nc.sync.dma_start(dbg_h.ap(), dh[:])

                c1 = gact.tile([P, B], dt.float32)
                nc.vector.tensor_mul(c1[:], fo[:, 0:B], cT)
                nc.vector.tensor_add(cT, c1[:], g_t[:])
                nc.vector.tensor_mul(hT[:, npar, 0, :], fo[:, ds(B, B)], cT)
                # prep the 7 outgoing writes (after the h write so the
                # trigger inherits the RAW edge on slot 0), then fire.
                # A gpsimd nop carries (a) the order-after-previous-trigger
                # wait and (b) the descriptor-ring reclaim wait.
                gnop = nc.gpsimd.nop()
                add_dep_helper(gnop.ins, prev_trig.ins, sync=False,
                               reason="preps after previous trigger")
                gnop._wait_ge(tsem, t + 1)
                if t >= 4:
                    gnop2 = nc.gpsimd.nop()
                    add_dep_helper(gnop2.ins, gnop.ins, sync=False,
                                   reason="ring reclaim nop chain")
                    post_waits.append((gnop2, lsem[t % 4], 112 * (t // 4)))
                    gnop = gnop2
                for k in range(1, NCORES):
                    rd = [None] * 8
                    rd[k] = (0, k)
                    pr = nc.gpsimd.remote_dma_broadcast(
                        hT[:, npar, k, :], hT[:, npar, 0, :],
                        rsem[npar], lsem[t % 4], rdests=rd)
                    if k == 1:
                        add_dep_helper(pr.ins, gnop.ins, sync=False,
                                       reason="preps after gate nop")
                prev_trig = nc.gpsimd.trigger_dma(count=None).then_inc(tsem)

                hf = ysb.tile([P, B], dt.float32)
                nc.vector.tensor_mul(hf[:], fo[:, ds(B, B)], cT)
                yp = yps.tile([B, P], dt.float32)
                nc.tensor.transpose(yp[:], hf[:], ident)
                yo = ysb.tile([B, P], dt.float32)
                nc.scalar.activation(yo[:], yp[:], AF.Copy)
                nc.sync.dma_start(y.ap()[:, t, :], yo[:])
                if t == s_steps - 1:
                    nc.sync.dma_start(hc.ap()[0], yo[:])
                    cp = yps.tile([B, P], dt.float32)
                    nc.tensor.transpose(cp[:], cT, ident)
                    co = ysb.tile([B, P], dt.float32)
                    nc.scalar.activation(co[:], cp[:], AF.Copy)
                    nc.sync.dma_start(hc.ap()[1], co[:])

        # stamp out (late, off the critical path)
        with tc.tile_pool(name="sto", bufs=1) as sto:
            so = sto.tile([P, NCORES, 8], dt.float32)
            vnop = nc.vector.nop()
            post_waits.append((vnop, ssem, 14))
            cp = nc.vector.tensor_copy(so[:], stp)
            add_dep_helper(cp.ins, vnop.ins, sync=False,
                           reason="stamp copy after stamp exchange")
            nc.sync.dma_start(stamp_o.ap(), so[:])

    for ins, sem, val in post_waits:
        ins._wait_ge(sem, val)
    nc.compile()
    return nc


def prep_shared(x, s_steps: int = S):
    # xT[j, p, t*B + b] = x[b, t, j*P + p]
    xt = np.ascontiguousarray(np.transpose(x[:, :s_steps, :], (2, 1, 0)))
    return {"xT": xt.astype(BF16).reshape(KT, P, s_steps * B)}


def prep_core(inputs, c, m_row):
    ws, us, bs = [], [], []
    rows = slice(c * HS, (c + 1) * HS)
    for g in "foc":
        Wg = inputs[f"W_{g}"][rows]          # [128, I]
        ws.append(np.stack([np.ascontiguousarray(Wg[:, j * P:(j + 1) * P].T)
                            for j in range(KT)]))
        Ug = inputs[f"U_{g}"][rows]          # [128, H]
        us.append(np.stack(
            [np.ascontiguousarray(Ug[:, m_row[k] * P:(m_row[k] + 1) * P].T)
             for k in range(NCORES)]))
        bs.append(inputs[f"b_{g}"][rows])
    # [G, KT, P(p), P(q)] -> SBUF layout [P(p), G, KT, P(q)]
    return {
        "wT": np.ascontiguousarray(
            np.transpose(np.stack(ws), (2, 0, 1, 3))).astype(BF16),
        "uT": np.ascontiguousarray(
            np.transpose(np.stack(us), (2, 0, 1, 3))).astype(BF16),
        "bias": np.stack(bs, axis=1).astype(np.float32),
        "idv": np.full((P, 8), float(c), np.float32),
        "ident": np.eye(P, dtype=np.float32),
    }


_cache = {}


def _get_program(s_steps):
    if s_steps not in _cache:
        _cache[s_steps] = build_program(s_steps)
    return _cache[s_steps]


def run_lstm(inputs, s_steps: int = S, m_map=None, _retried=False):
    m_map = [list(r) for r in (m_map or M_MAP)]
    nc = _get_program(s_steps)
    shared = prep_shared(np.asarray(inputs["x"], np.float32), s_steps)
    in_maps = []
    for c in range(NCORES):
        m = dict(shared)
        m.update(prep_core(inputs, c, m_map[c]))
        in_maps.append(m)
    res = bass_utils.run_bass_kernel_spmd(nc, in_maps,
                                          core_ids=list(range(NCORES)))
    # validate slot ownership against the observed stamps
    observed = []
    ok = True
    for c in range(NCORES):
        st = res.results[c]["stamp"]
        row = [int(round(float(st[0, k, 0]))) for k in range(NCORES)]
        observed.append(row)
        if row != m_map[c]:
            ok = False
    if not ok:
        if _retried:
            raise RuntimeError(f"slot map unstable: {observed}")
        return run_lstm(inputs, s_steps, m_map=observed, _retried=True)

    yf = np.concatenate([res.results[c]["y"] for c in range(NCORES)], axis=2)
    hf = np.concatenate([res.results[c]["hc"][0] for c in range(NCORES)],
                        axis=1)[None]
    cf = np.concatenate([res.results[c]["hc"][1] for c in range(NCORES)],
                        axis=1)[None]
    return yf, hf, cf


def kernel(**inputs):
    return run_lstm(inputs, S)


def bench(inputs, s_steps: int = S, iters: int = 12):
    """Estimate device time by pipelining `iters` executions of the cached
    PJRT executable and timing the drain. Returns ns per execution."""
    import time
    import jax
    import jax.numpy as jnp
    from jax.sharding import Mesh, PartitionSpec
    from jax.experimental.shard_map import shard_map
    from concourse import bass2jax as b2j

    nc = _get_program(s_steps)
    b2j.install_neuronx_cc_hook()
    shared = prep_shared(np.asarray(inputs["x"], np.float32), s_steps)
    in_maps = []
    for c in range(NCORES):
        m = dict(shared)
        m.update(prep_core(inputs, c, M_MAP[c]))
        in_maps.append(m)

    import concourse.mybir as mybir
    partition_name = (nc.partition_id_tensor.name
                      if nc.partition_id_tensor else None)
    in_names, out_names, out_avals, zero_outs = [], [], [], []
    for alloc in nc.m.functions[0].allocations:
        if not isinstance(alloc, mybir.MemoryLocationSet):
            continue
        name = alloc.memorylocations[0].name
        if alloc.kind == "ExternalInput":
            if name != partition_name:
                in_names.append(name)
        elif alloc.kind == "ExternalOutput":
            shape = tuple(alloc.tensor_shape)
            dtype = mybir.dt.np(alloc.dtype)
            out_names.append(name)
            out_avals.append(jax.core.ShapedArray(shape, dtype))
            zero_outs.append(np.zeros(shape, dtype))
    n_params = len(in_names)
    all_names = list(in_names) + list(out_names)
    if partition_name is not None:
        all_names.append(partition_name)

    def _body(*args):
        operands = list(args)
        if partition_name is not None:
            operands.append(b2j.partition_id_tensor())
        return tuple(b2j._bass_exec_p.bind(
            *operands, out_avals=tuple(out_avals), in_names=tuple(all_names),
            out_names=tuple(out_names), lowering_input_output_aliases=(),
            sim_require_finite=False, sim_require_nnan=False, nc=nc))

    devices = jax.devices()[:NCORES]
    mesh = Mesh(np.asarray(devices), ("core",))
    nin = n_params + len(out_names)
    fn = jax.jit(shard_map(_body, mesh=mesh,
                           in_specs=(PartitionSpec("core"),) * nin,
                           out_specs=(PartitionSpec("core"),) * len(out_names),
                           check_rep=False), keep_unused=True)
    concat_in = [np.concatenate([np.asarray(in_maps[c][n])[None]
                                 for c in range(NCORES)]).reshape(
                     NCORES * np.asarray(in_maps[0][n]).shape[0],
                     *np.asarray(in_maps[0][n]).shape[1:])
                 for n in in_names]
    concat_zo = [np.concatenate([z[None]] * NCORES).reshape(
        NCORES * z.shape[0], *z.shape[1:]) for z in zero_outs]
    args = [jax.device_put(a) for a in concat_in + concat_zo]
    # warmup
    r = fn(*args)
    jax.block_until_ready(r)
    t0 = time.time()
    rs = [fn(*args) for _ in range(iters)]
    jax.block_until_ready(rs)
    t1 = time.time()
    return (t1 - t0) / iters * 1e9


if __name__ == "__main__":
    rng = np.random.default_rng(0)
    stdv = 1.0 / np.sqrt(H)
    demo = {"x": rng.standard_normal((B, S, I), dtype=np.float32)}
    for g in "fioc":
        demo[f"W_{g}"] = rng.uniform(-stdv, stdv, (H, I)).astype(np.float32)
        demo[f"U_{g}"] = rng.uniform(-stdv, stdv, (H, H)).astype(np.float32)
        demo[f"b_{g}"] = rng.uniform(-stdv, stdv, (H,)).astype(np.float32)
    out, h, c = run_lstm(demo, 16)
    print("ran", out.shape, h.shape, c.shape)


# revision 53
# speedup vs baseline: 2.4460x; 2.4460x over previous
"""GatedLSTM Trainium2 kernel: 8-core hidden-sharded recurrence.

Strategy
--------
B=64, S=512, I=H=1024. Gates f, o, c (input gate `i` is unused by the
reference). Work is split across the 8 NeuronCores by the *hidden* dim:
core c owns rows [128c, 128c+128) of all three gates. This keeps every
per-step matmul at full PE width (stationary U tiles are 128x128) instead
of the batch-parallel split, which would leave the PE array 8/128 occupied
and re-stream all of U through every core each step.

Phase 1 (parallel): X_g = x @ W_g.T + b_g for the core's 128-row slice,
computed as 8 K-tile matmuls per 512-column chunk from a pre-transposed
bf16 copy of x, written to DRAM scratch in the step-major layout phase 2
consumes.

Phase 2 (recurrent, 512 steps): each step does 24 bf16 matmuls
(3 gates x 8 K-tiles) of U-tile[128,128] @ hT[128,64] accumulating in
PSUM, adds the X slice (DVE), applies sigmoid/tanh (ACT), updates c and h
(DVE), then broadcasts its 16KB hT slice to the other 7 cores through
`remote_dma_broadcast` (SWDGE P2P SBUF->SBUF writes with remote semaphore
increments). Waits: each peer's broadcast bumps the receiver's parity
semaphore by 2, so a step waits for +14 since the previous same-parity
exchange. The h state buffer is double buffered by step parity.

Cross-core slot mapping: relative XOR routing lands sender s in slot k of
receiver r with tpb(s) = tpb(r) XOR k. The device-order -> tpb mapping of
this container was probed; M_MAP[c][k] below is "which device's h slice
lives in slot k on device c". Per-core U inputs are permuted accordingly
on the host. A stamp exchange validates the map at runtime; on mismatch
the kernel rebuilds the permutation from the observed stamps and reruns.
"""

import numpy as np
import ml_dtypes

import concourse.bass as bass
import concourse.bacc as bacc
import concourse.tile as tile
import concourse.mybir as mybir
from concourse import bass_utils, library_config
from concourse.bass import ds, ts
from concourse.tile_rust import add_dep_helper

B, S, I, H = 64, 512, 1024, 1024
P = 128
NCORES = 8
HS = H // NCORES          # 128 hidden rows per core
KT = H // P               # 8 contraction K-tiles
G = 3                     # gates packed [f | o | c]
CH = 512                  # phase-1 chunk: 8 steps x 64 batch columns
AF = mybir.ActivationFunctionType

# slot k on device c holds the h slice owned by device M_MAP[c][k]
# (probed on this container: devices 4..7 sit on tpbs 6,7,4,5).
M_MAP = [
    [0, 1, 2, 3, 6, 7, 4, 5],
    [1, 0, 3, 2, 7, 6, 5, 4],
    [2, 3, 0, 1, 4, 5, 6, 7],
    [3, 2, 1, 0, 5, 4, 7, 6],
    [4, 5, 6, 7, 2, 3, 0, 1],
    [5, 4, 7, 6, 3, 2, 1, 0],
    [6, 7, 4, 5, 0, 1, 2, 3],
    [7, 6, 5, 4, 1, 0, 3, 2],
]

BF16 = ml_dtypes.bfloat16


def build_program(s_steps: int = S, debug: bool = False, s_in: int = None,
                  no_comm: bool = False):
    assert s_steps % 8 == 0
    s_in = s_in or s_steps
    dt = mybir.dt
    nc = bacc.Bacc("TRN2", target_bir_lowering=False, debug=False,
                   num_devices=NCORES)

    xT = nc.dram_tensor("xT", [KT, P, s_in * B], dt.bfloat16,
                        kind="ExternalInput")
    # weight tensors are staged in SBUF-layout: [partition, gate, ktile, col]
    wT = nc.dram_tensor("wT", [P, G, KT, P], dt.bfloat16, kind="ExternalInput")
    uT = nc.dram_tensor("uT", [P, G, KT, P], dt.bfloat16, kind="ExternalInput")
    bia = nc.dram_tensor("bias", [P, G], dt.float32, kind="ExternalInput")
    idv = nc.dram_tensor("idv", [P, 8], dt.float32, kind="ExternalInput")
    idn = nc.dram_tensor("ident", [P, P], dt.float32, kind="ExternalInput")

    y = nc.dram_tensor("y", [B, s_steps, HS], dt.float32, kind="ExternalOutput")
    hc = nc.dram_tensor("hc", [2, B, HS], dt.float32, kind="ExternalOutput")
    stamp_o = nc.dram_tensor("stamp", [P, NCORES, 8], dt.float32,
                             kind="ExternalOutput")

    Xpre = nc.dram_tensor("Xpre", [P, s_steps, G * B], dt.float32,
                          kind="Internal")
    if debug:
        dbg_x = nc.dram_tensor("dbg_x", [P, G * B], dt.float32,
                               kind="ExternalOutput")
        dbg_pre = nc.dram_tensor("dbg_pre", [P, G * B], dt.float32,
                                 kind="ExternalOutput")
        dbg_h = nc.dram_tensor("dbg_h", [P, NCORES, B], dt.float32,
                               kind="ExternalOutput")

    # waits on semaphores that only remote cores increment are attached
    # *after* Tile scheduling: the scheduler's single-core sim cannot model
    # remote increments and would report a false deadlock.
    post_waits = []

    with tile.TileContext(nc) as tc:
        u_sb = nc.alloc_sbuf_tensor("u_sb", [P, G, KT, P], dt.bfloat16).ap()
        w_sb = nc.alloc_sbuf_tensor("w_sb", [P, G, KT, P], dt.bfloat16).ap()
        b_sb = nc.alloc_sbuf_tensor("b_sb", [P, G], dt.float32).ap()
        hT = nc.alloc_sbuf_tensor("hT", [P, 2, NCORES, B], dt.bfloat16).ap()
        cT = nc.alloc_sbuf_tensor("cT", [P, B], dt.float32).ap()
        stp = nc.alloc_sbuf_tensor("stp", [P, NCORES, 8], dt.float32).ap()
        ident = nc.alloc_sbuf_tensor("identity", [P, P], dt.float32).ap()

        rsem = [nc.alloc_semaphore("rsem0"), nc.alloc_semaphore("rsem1")]
        ssem = nc.alloc_semaphore("ssem")
        lsem = [nc.alloc_semaphore(f"lsem{i}") for i in range(4)]
        lsem_s = nc.alloc_semaphore("lsem_s")
        tsem = nc.alloc_semaphore("tsem")   # orders preps after prior trigger
        xsem = nc.alloc_semaphore("xsem")   # phase1 -> phase2 Xpre handoff

        nc.gpsimd.load_library(library_config.remote_dma)

        nc.sync.dma_start(u_sb, uT.ap())
        nc.sync.dma_start(w_sb, wT.ap())
        nc.sync.dma_start(b_sb, bia.ap())
        nc.sync.dma_start(ident, idn.ap())
        nc.sync.dma_start(stp[:, 0, :], idv.ap())
        nc.vector.memset(hT[:, 0], 0.0)
        nc.vector.memset(cT, 0.0)

        # stamp exchange: validates the device->slot ownership map
        if not no_comm:
            for k in range(1, NCORES):
                rd = [None] * 8
                rd[k] = (0, k)
                nc.gpsimd.remote_dma_broadcast(stp[:, k, :], stp[:, 0, :],
                                               ssem, lsem_s, rdests=rd)
            prev_trig = nc.gpsimd.trigger_dma(count=None).then_inc(tsem)

        # ---------------- phase 1: X = x @ W.T + b ----------------
        nch = s_steps * B // CH
        with tc.tile_pool(name="xc", bufs=3) as xpool, \
             tc.tile_pool(name="p1ps", bufs=3, space="PSUM") as p1ps, \
             tc.tile_pool(name="p1o", bufs=3) as p1o:
            for ci in range(nch):
                xt = xpool.tile([P, KT, CH], dt.bfloat16)
                for j in range(KT):
                    nc.sync.dma_start(xt[:, j], xT.ap()[j, :, ts(ci, CH)])
                x_outs = []
                for g in range(G):
                    ps = p1ps.tile([P, CH], dt.float32)
                    for j in range(KT):
                        nc.tensor.matmul(ps[:], w_sb[:, g, j], xt[:, j],
                                         start=(j == 0), stop=(j == KT - 1))
                    xo = p1o.tile([P, 8, B], dt.float32)
                    nc.scalar.activation(xo[:], ps[:].rearrange("p (t b) -> p t b", b=B),
                                         AF.Identity, bias=b_sb[:, ds(g, 1)])
                    x_outs.append(nc.sync.dma_start(
                        Xpre.ap()[:, ts(ci, 8), ts(g, B)], xo[:]))
                dnop = nc.vector.nop()
                for o in x_outs:
                    add_dep_helper(dnop.ins, o.ins, sync=True,
                                   reason="Xpre chunk stores complete")
                dnop.then_inc(xsem)

        # ---------------- phase 2: recurrence ----------------
        with tc.tile_pool(name="xq", bufs=8) as xq, \
             tc.tile_pool(name="p2ps", bufs=2, space="PSUM") as p2ps, \
             tc.tile_pool(name="gact", bufs=3) as gact, \
             tc.tile_pool(name="yps", bufs=2, space="PSUM") as yps, \
             tc.tile_pool(name="ysb", bufs=4) as ysb:
            for t in range(s_steps):
                par = t % 2
                npar = 1 - par
                xt_t = xq.tile([P, G * B], dt.float32)
                xnop = nc.vector.nop()
                xnop._wait_ge(xsem, t // 8 + 1)
                xload = nc.sync.dma_start(xt_t[:], Xpre.ap()[:, t, :])
                add_dep_helper(xload.ins, xnop.ins, sync=True,
                               reason="X load after phase-1 chunk")

                ps_fo = p2ps.tile([P, 2 * B], dt.float32)
                ps_cc = p2ps.tile([P, B], dt.float32)
                wait_val = 14 * ((t + 1) // 2)
                # The receive-wait rides on a PE nop anchored after a k=0
                # matmul (k=0 reads the locally written slot 0, which RAW-
                # chains it behind the whole previous step) and before every
                # gate's k=1 matmul (the first reader of remote slots).
                rnop = None
                for g in (2, 0, 1):   # c first so the tanh path starts early
                    dst = ps_cc[:] if g == 2 else ps_fo[:, ts(g, B)]
                    for k in range(NCORES):
                        mm = nc.tensor.matmul(dst, u_sb[:, g, k],
                                              hT[:, par, k, :],
                                              start=(k == 0),
                                              stop=(k == NCORES - 1))
                        if t > 0 and not no_comm:
                            if k == 0 and g == 2:
                                rnop = nc.tensor.nop()
                                add_dep_helper(rnop.ins, mm.ins, sync=False,
                                               reason="rwait after local k0")
                                post_waits.append((rnop, rsem[par], wait_val))
                            elif k == 1:
                                add_dep_helper(mm.ins, rnop.ins, sync=False,
                                               reason="remote slots gated")

                pre_cc = gact.tile([P, B], dt.float32)
                nc.vector.tensor_add(pre_cc[:], ps_cc[:], xt_t[:, ds(2 * B, B)])
                g_t = gact.tile([P, B], dt.float32)
                nc.scalar.activation(g_t[:], pre_cc[:], AF.Tanh)

                pre_fo = gact.tile([P, 2 * B], dt.float32)
                nc.vector.tensor_add(pre_fo[:], ps_fo[:], xt_t[:, 0:2 * B])
                fo = gact.tile([P, 2 * B], dt.float32)
                nc.scalar.activation(fo[:], pre_fo[:], AF.Sigmoid)

                if debug and t == 1:
                    nc.sync.dma_start(dbg_x.ap(), xt_t[:])
                    dpre = gact.tile([P, G * B], dt.float32)
                    nc.vector.tensor_copy(dpre[:, 0:2 * B], pre_fo[:])
                    nc.vector.tensor_copy(dpre[:, ds(2 * B, B)], pre_cc[:])
                    nc.sync.dma_start(dbg_pre.ap(), dpre[:])
                    dh = gact.tile([P, NCORES, B], dt.float32)
                    dcp = nc.vector.tensor_copy(dh[:], hT[:, par])
                    add_dep_helper(dcp.ins, rnop.ins, sync=True,
                                   reason="dbg after receive wait")
                    nc.sync.dma_start(dbg_h.ap(), dh[:])

                c1 = gact.tile([P, B], dt.float32)
                nc.vector.tensor_mul(c1[:], fo[:, 0:B], cT)
                nc.vector.tensor_add(cT, c1[:], g_t[:])
                nc.vector.tensor_mul(hT[:, npar, 0, :], fo[:, ds(B, B)], cT)
                if not no_comm:
                    gnop = nc.gpsimd.nop()
                    add_dep_helper(gnop.ins, prev_trig.ins, sync=False,
                                   reason="preps after previous trigger")
                    gnop._wait_ge(tsem, t + 1)
                    if t >= 4:
                        gnop2 = nc.gpsimd.nop()
                        add_dep_helper(gnop2.ins, gnop.ins, sync=False,
                                       reason="ring reclaim nop chain")
                        post_waits.append((gnop2, lsem[t % 4],
                                           112 * (t // 4)))
                        gnop = gnop2
                    for k in range(1, NCORES):
                        rd = [None] * 8
                        rd[k] = (0, k)
                        pr = nc.gpsimd.remote_dma_broadcast(
                            hT[:, npar, k, :], hT[:, npar, 0, :],
                            rsem[npar], lsem[t % 4], rdests=rd)
                        if k == 1:
                            add_dep_helper(pr.ins, gnop.ins, sync=False,
                                           reason="preps after gate nop")
                    prev_trig = nc.gpsimd.trigger_dma(
                        count=None).then_inc(tsem)

                hf = ysb.tile([P, B], dt.float32)
                nc.vector.tensor_mul(hf[:], fo[:, ds(B, B)], cT)
                yp = yps.tile([B, P], dt.float32)
                nc.tensor.transpose(yp[:], hf[:], ident)
                yo = ysb.tile([B, P], dt.float32)
                nc.scalar.activation(yo[:], yp[:], AF.Copy)
                nc.sync.dma_start(y.ap()[:, t, :], yo[:])
                if t == s_steps - 1:
                    nc.sync.dma_start(hc.ap()[0], yo[:])
                    cp = yps.tile([B, P], dt.float32)
                    nc.tensor.transpose(cp[:], cT, ident)
                    co = ysb.tile([B, P], dt.float32)
                    nc.scalar.activation(co[:], cp[:], AF.Copy)
                    nc.sync.dma_start(hc.ap()[1], co[:])

        # stamp out (late, off the critical path)
        with tc.tile_pool(name="sto", bufs=1) as sto:
            so = sto.tile([P, NCORES, 8], dt.float32)
            cp = nc.vector.tensor_copy(so[:], stp)
            if not no_comm:
                vnop = nc.vector.nop()
                post_waits.append((vnop, ssem, 14))
                add_dep_helper(cp.ins, vnop.ins, sync=False,
                               reason="stamp copy after stamp exchange")
            nc.sync.dma_start(stamp_o.ap(), so[:])

    for ins, sem, val in post_waits:
        ins._wait_ge(sem, val)
    nc.compile()
    return nc


def prep_shared(x, s_steps: int = S):
    # xT[j, p, t*B + b] = x[b, t, j*P + p]
    xt = np.ascontiguousarray(np.transpose(x[:, :s_steps, :], (2, 1, 0)))
    return {"xT": xt.astype(BF16).reshape(KT, P, s_steps * B)}


def prep_core(inputs, c, m_row):
    ws, us, bs = [], [], []
    rows = slice(c * HS, (c + 1) * HS)
    for g in "foc":
        Wg = inputs[f"W_{g}"][rows]          # [128, I]
        ws.append(np.stack([np.ascontiguousarray(Wg[:, j * P:(j + 1) * P].T)
                            for j in range(KT)]))
        Ug = inputs[f"U_{g}"][rows]          # [128, H]
        us.append(np.stack(
            [np.ascontiguousarray(Ug[:, m_row[k] * P:(m_row[k] + 1) * P].T)
             for k in range(NCORES)]))
        bs.append(inputs[f"b_{g}"][rows])
    # [G, KT, P(p), P(q)] -> SBUF layout [P(p), G, KT, P(q)]
    return {
        "wT": np.ascontiguousarray(
            np.transpose(np.stack(ws), (2, 0, 1, 3))).astype(BF16),
        "uT": np.ascontiguousarray(
            np.transpose(np.stack(us), (2, 0, 1, 3))).astype(BF16),
        "bias": np.stack(bs, axis=1).astype(np.float32),
        "idv": np.full((P, 8), float(c), np.float32),
        "ident": np.eye(P, dtype=np.float32),
    }


_cache = {}


def _get_program(s_steps, s_in=None, no_comm=False):
    key = (s_steps, s_in, no_comm)
    if key not in _cache:
        _cache[key] = build_program(s_steps, s_in=s_in, no_comm=no_comm)
    return _cache[key]


def run_lstm(inputs, s_steps: int = S, m_map=None, _retried=False):
    m_map = [list(r) for r in (m_map or M_MAP)]
    nc = _get_program(s_steps)
    shared = prep_shared(np.asarray(inputs["x"], np.float32), s_steps)
    in_maps = []
    for c in range(NCORES):
        m = dict(shared)
        m.update(prep_core(inputs, c, m_map[c]))
        in_maps.append(m)
    res = bass_utils.run_bass_kernel_spmd(nc, in_maps,
                                          core_ids=list(range(NCORES)))
    # validate slot ownership against the observed stamps
    observed = []
    ok = True
    for c in range(NCORES):
        st = res.results[c]["stamp"]
        row = [int(round(float(st[0, k, 0]))) for k in range(NCORES)]
        observed.append(row)
        if row != m_map[c]:
            ok = False
    if not ok:
        if _retried:
            raise RuntimeError(f"slot map unstable: {observed}")
        return run_lstm(inputs, s_steps, m_map=observed, _retried=True)

    yf = np.concatenate([res.results[c]["y"] for c in range(NCORES)], axis=2)
    hf = np.concatenate([res.results[c]["hc"][0] for c in range(NCORES)],
                        axis=1)[None]
    cf = np.concatenate([res.results[c]["hc"][1] for c in range(NCORES)],
                        axis=1)[None]
    return yf, hf, cf


def kernel(**inputs):
    return run_lstm(inputs, S)


def bench(inputs, s_steps: int = S, iters: int = 12, s_in: int = None,
          no_comm: bool = False):
    """Estimate device time by pipelining `iters` executions of the cached
    PJRT executable and timing the drain. Returns ns per execution."""
    import time
    import jax
    import jax.numpy as jnp
    from jax.sharding import Mesh, PartitionSpec
    from jax.experimental.shard_map import shard_map
    from concourse import bass2jax as b2j

    nc = _get_program(s_steps, s_in, no_comm)
    b2j.install_neuronx_cc_hook()
    shared = prep_shared(np.asarray(inputs["x"], np.float32), s_in or s_steps)
    in_maps = []
    for c in range(NCORES):
        m = dict(shared)
        m.update(prep_core(inputs, c, M_MAP[c]))
        in_maps.append(m)

    import concourse.mybir as mybir
    partition_name = (nc.partition_id_tensor.name
                      if nc.partition_id_tensor else None)
    in_names, out_names, out_avals, zero_outs = [], [], [], []
    for alloc in nc.m.functions[0].allocations:
        if not isinstance(alloc, mybir.MemoryLocationSet):
            continue
        name = alloc.memorylocations[0].name
        if alloc.kind == "ExternalInput":
            if name != partition_name:
                in_names.append(name)
        elif alloc.kind == "ExternalOutput":
            shape = tuple(alloc.tensor_shape)
            dtype = mybir.dt.np(alloc.dtype)
            out_names.append(name)
            out_avals.append(jax.core.ShapedArray(shape, dtype))
            zero_outs.append(np.zeros(shape, dtype))
    n_params = len(in_names)
    all_names = list(in_names) + list(out_names)
    if partition_name is not None:
        all_names.append(partition_name)

    def _body(*args):
        operands = list(args)
        if partition_name is not None:
            operands.append(b2j.partition_id_tensor())
        return tuple(b2j._bass_exec_p.bind(
            *operands, out_avals=tuple(out_avals), in_names=tuple(all_names),
            out_names=tuple(out_names), lowering_input_output_aliases=(),
            sim_require_finite=False, sim_require_nnan=False, nc=nc))

    devices = jax.devices()[:NCORES]
    mesh = Mesh(np.asarray(devices), ("core",))
    nin = n_params + len(out_names)
    fn = jax.jit(shard_map(_body, mesh=mesh,
                           in_specs=(PartitionSpec("core"),) * nin,
                           out_specs=(PartitionSpec("core"),) * len(out_names),
                           check_rep=False), keep_unused=True)
    concat_in = [np.concatenate([np.asarray(in_maps[c][n])[None]
                                 for c in range(NCORES)]).reshape(
                     NCORES * np.asarray(in_maps[0][n]).shape[0],
                     *np.asarray(in_maps[0][n]).shape[1:])
                 for n in in_names]
    concat_zo = [np.concatenate([z[None]] * NCORES).reshape(
        NCORES * z.shape[0], *z.shape[1:]) for z in zero_outs]
    args = [jax.device_put(a) for a in concat_in + concat_zo]
    # warmup
    r = fn(*args)
    jax.block_until_ready(r)
    t0 = time.time()
    rs = [fn(*args) for _ in range(iters)]
    jax.block_until_ready(rs)
    t1 = time.time()
    return (t1 - t0) / iters * 1e9


if __name__ == "__main__":
    rng = np.random.default_rng(0)
    stdv = 1.0 / np.sqrt(H)
    demo = {"x": rng.standard_normal((B, S, I), dtype=np.float32)}
    for g in "fioc":
        demo[f"W_{g}"] = rng.uniform(-stdv, stdv, (H, I)).astype(np.float32)
        demo[f"U_{g}"] = rng.uniform(-stdv, stdv, (H, H)).astype(np.float32)
        demo[f"b_{g}"] = rng.uniform(-stdv, stdv, (H,)).astype(np.float32)
    out, h, c = run_lstm(demo, 16)
    print("ran", out.shape, h.shape, c.shape)


# revision 63
# speedup vs baseline: 2.7689x; 1.1320x over previous
"""GatedLSTM Trainium2 kernel: 8-core hidden-sharded recurrence.

Strategy
--------
B=64, S=512, I=H=1024. Gates f, o, c (input gate `i` is unused by the
reference). Work is split across the 8 NeuronCores by the *hidden* dim:
core c owns rows [128c, 128c+128) of all three gates. This keeps every
per-step matmul at full PE width (stationary U tiles are 128x128) instead
of the batch-parallel split, which would leave the PE array 8/128 occupied
and re-stream all of U through every core each step.

Phase 1 (parallel): X_g = x @ W_g.T + b_g for the core's 128-row slice,
computed as 8 K-tile matmuls per 512-column chunk from a pre-transposed
bf16 copy of x, written to DRAM scratch in the step-major layout phase 2
consumes.

Phase 2 (recurrent, 512 steps): each step does 24 bf16 matmuls
(3 gates x 8 K-tiles) of U-tile[128,128] @ hT[128,64] accumulating in
PSUM, adds the X slice (DVE), applies sigmoid/tanh (ACT), updates c and h
(DVE), then broadcasts its 16KB hT slice to the other 7 cores through
`remote_dma_broadcast` (SWDGE P2P SBUF->SBUF writes with remote semaphore
increments). Waits: each peer's broadcast bumps the receiver's parity
semaphore by 2, so a step waits for +14 since the previous same-parity
exchange. The h state buffer is double buffered by step parity.

Cross-core slot mapping: relative XOR routing lands sender s in slot k of
receiver r with tpb(s) = tpb(r) XOR k. The device-order -> tpb mapping of
this container was probed; M_MAP[c][k] below is "which device's h slice
lives in slot k on device c". Per-core U inputs are permuted accordingly
on the host. A stamp exchange validates the map at runtime; on mismatch
the kernel rebuilds the permutation from the observed stamps and reruns.
"""

import numpy as np
import ml_dtypes

import concourse.bass as bass
import concourse.bacc as bacc
import concourse.tile as tile
import concourse.mybir as mybir
from concourse import bass_utils, library_config
from concourse.bass import ds, ts
from concourse.tile_rust import add_dep_helper

B, S, I, H = 64, 512, 1024, 1024
P = 128
NCORES = 8
HS = H // NCORES          # 128 hidden rows per core
KT = H // P               # 8 contraction K-tiles
G = 3                     # gates packed [f | o | c]
CH = 512                  # phase-1 chunk: 8 steps x 64 batch columns
AF = mybir.ActivationFunctionType

# slot k on device c holds the h slice owned by device M_MAP[c][k]
# (probed on this container: devices 4..7 sit on tpbs 6,7,4,5).
M_MAP = [
    [0, 1, 2, 3, 6, 7, 4, 5],
    [1, 0, 3, 2, 7, 6, 5, 4],
    [2, 3, 0, 1, 4, 5, 6, 7],
    [3, 2, 1, 0, 5, 4, 7, 6],
    [4, 5, 6, 7, 2, 3, 0, 1],
    [5, 4, 7, 6, 3, 2, 1, 0],
    [6, 7, 4, 5, 0, 1, 2, 3],
    [7, 6, 5, 4, 1, 0, 3, 2],
]

BF16 = ml_dtypes.bfloat16


def build_program(s_steps: int = S, debug: bool = False, s_in: int = None,
                  no_comm: bool = False, no_wait: bool = False):
    assert s_steps % 8 == 0
    s_in = s_in or s_steps
    dt = mybir.dt
    nc = bacc.Bacc("TRN2", target_bir_lowering=False, debug=False,
                   num_devices=NCORES)

    xT = nc.dram_tensor("xT", [KT, P, s_in * B], dt.bfloat16,
                        kind="ExternalInput")
    # weight tensors are staged in SBUF-layout: [partition, gate, ktile, col]
    wT = nc.dram_tensor("wT", [P, G, KT, P], dt.bfloat16, kind="ExternalInput")
    uT = nc.dram_tensor("uT", [P, G, KT, P], dt.bfloat16, kind="ExternalInput")
    bia = nc.dram_tensor("bias", [P, G], dt.float32, kind="ExternalInput")
    idv = nc.dram_tensor("idv", [P, 8], dt.float32, kind="ExternalInput")
    idn = nc.dram_tensor("ident", [P, P], dt.float32, kind="ExternalInput")

    y = nc.dram_tensor("y", [B, s_steps, HS], dt.float32, kind="ExternalOutput")
    hc = nc.dram_tensor("hc", [2, B, HS], dt.float32, kind="ExternalOutput")
    stamp_o = nc.dram_tensor("stamp", [P, NCORES, 8], dt.float32,
                             kind="ExternalOutput")

    Xpre = nc.dram_tensor("Xpre", [P, s_steps, G * B], dt.float32,
                          kind="Internal")
    if debug:
        dbg_x = nc.dram_tensor("dbg_x", [P, G * B], dt.float32,
                               kind="ExternalOutput")
        dbg_pre = nc.dram_tensor("dbg_pre", [P, G * B], dt.float32,
                                 kind="ExternalOutput")
        dbg_h = nc.dram_tensor("dbg_h", [P, NCORES, B], dt.float32,
                               kind="ExternalOutput")

    # waits on semaphores that only remote cores increment are attached
    # *after* Tile scheduling: the scheduler's single-core sim cannot model
    # remote increments and would report a false deadlock.
    post_waits = []

    with tile.TileContext(nc) as tc:
        u_sb = nc.alloc_sbuf_tensor("u_sb", [P, G, KT, P], dt.bfloat16).ap()
        w_sb = nc.alloc_sbuf_tensor("w_sb", [P, G, KT, P], dt.bfloat16).ap()
        b_sb = nc.alloc_sbuf_tensor("b_sb", [P, G], dt.float32).ap()
        hT = nc.alloc_sbuf_tensor("hT", [P, 2, NCORES, B], dt.bfloat16).ap()
        cT = nc.alloc_sbuf_tensor("cT", [P, B], dt.float32).ap()
        stp = nc.alloc_sbuf_tensor("stp", [P, NCORES, 8], dt.float32).ap()
        ident = nc.alloc_sbuf_tensor("identity", [P, P], dt.float32).ap()

        rsem = [nc.alloc_semaphore("rsem0"), nc.alloc_semaphore("rsem1")]
        ssem = nc.alloc_semaphore("ssem")
        lsem = [nc.alloc_semaphore(f"lsem{i}") for i in range(4)]
        lsem_s = nc.alloc_semaphore("lsem_s")
        tsem = nc.alloc_semaphore("tsem")   # orders preps after prior trigger
        xsem = nc.alloc_semaphore("xsem")   # phase1 -> phase2 Xpre handoff

        nc.gpsimd.load_library(library_config.remote_dma)

        nc.sync.dma_start(u_sb, uT.ap())
        nc.sync.dma_start(w_sb, wT.ap())
        nc.sync.dma_start(b_sb, bia.ap())
        nc.sync.dma_start(ident, idn.ap())
        nc.sync.dma_start(stp[:, 0, :], idv.ap())
        nc.vector.memset(hT[:, 0], 0.0)
        nc.vector.memset(cT, 0.0)

        # stamp exchange: validates the device->slot ownership map
        if not no_comm:
            for k in range(1, NCORES):
                rd = [None] * 8
                rd[k] = (0, k)
                nc.gpsimd.remote_dma_broadcast(stp[:, k, :], stp[:, 0, :],
                                               ssem, lsem_s, rdests=rd)
            prev_trig = nc.gpsimd.trigger_dma(count=None).then_inc(tsem)

        # ---------------- phase 1: X = x @ W.T + b ----------------
        nch = s_steps * B // CH
        with tc.tile_pool(name="xc", bufs=3) as xpool, \
             tc.tile_pool(name="p1ps", bufs=3, space="PSUM") as p1ps, \
             tc.tile_pool(name="p1o", bufs=3) as p1o:
            for ci in range(nch):
                xt = xpool.tile([P, KT, CH], dt.bfloat16)
                for j in range(KT):
                    nc.sync.dma_start(xt[:, j], xT.ap()[j, :, ts(ci, CH)])
                x_outs = []
                for g in range(G):
                    ps = p1ps.tile([P, CH], dt.float32)
                    for j in range(KT):
                        nc.tensor.matmul(ps[:], w_sb[:, g, j], xt[:, j],
                                         start=(j == 0), stop=(j == KT - 1))
                    xo = p1o.tile([P, 8, B], dt.float32)
                    nc.scalar.activation(xo[:], ps[:].rearrange("p (t b) -> p t b", b=B),
                                         AF.Identity, bias=b_sb[:, ds(g, 1)])
                    x_outs.append(nc.sync.dma_start(
                        Xpre.ap()[:, ts(ci, 8), ts(g, B)], xo[:]))
                dnop = nc.vector.nop()
                for o in x_outs:
                    add_dep_helper(dnop.ins, o.ins, sync=True,
                                   reason="Xpre chunk stores complete")
                dnop.then_inc(xsem)

        # ---------------- phase 2: recurrence ----------------
        with tc.tile_pool(name="xq", bufs=8) as xq, \
             tc.tile_pool(name="p2ps", bufs=2, space="PSUM") as p2ps, \
             tc.tile_pool(name="gact", bufs=3) as gact, \
             tc.tile_pool(name="yps", bufs=2, space="PSUM") as yps, \
             tc.tile_pool(name="ysb", bufs=4) as ysb:
            for t in range(s_steps):
                par = t % 2
                npar = 1 - par
                xt_t = xq.tile([P, G * B], dt.float32)
                xnop = nc.vector.nop()
                xnop._wait_ge(xsem, t // 8 + 1)
                xload = nc.sync.dma_start(xt_t[:], Xpre.ap()[:, t, :])
                add_dep_helper(xload.ins, xnop.ins, sync=True,
                               reason="X load after phase-1 chunk")

                ps_fo = p2ps.tile([P, 2 * B], dt.float32)
                ps_cc = p2ps.tile([P, B], dt.float32)
                wait_val = 14 * ((t + 1) // 2)
                # The receive-wait rides on a PE nop anchored after a k=0
                # matmul (k=0 reads the locally written slot 0, which RAW-
                # chains it behind the whole previous step) and before every
                # gate's k=1 matmul (the first reader of remote slots).
                rnop = None
                for g in (2, 0, 1):   # c first so the tanh path starts early
                    dst = ps_cc[:] if g == 2 else ps_fo[:, ts(g, B)]
                    for k in range(NCORES):
                        mm = nc.tensor.matmul(dst, u_sb[:, g, k],
                                              hT[:, par, k, :],
                                              start=(k == 0),
                                              stop=(k == NCORES - 1))
                        if t > 0 and not no_comm and not no_wait:
                            if k == 0 and g == 2:
                                rnop = nc.tensor.nop()
                                add_dep_helper(rnop.ins, mm.ins, sync=False,
                                               reason="rwait after local k0")
                                post_waits.append((rnop, rsem[par], wait_val))
                            elif k == 1:
                                add_dep_helper(mm.ins, rnop.ins, sync=False,
                                               reason="remote slots gated")

                pre_cc = gact.tile([P, B], dt.float32)
                nc.vector.tensor_add(pre_cc[:], ps_cc[:], xt_t[:, ds(2 * B, B)])
                g_t = gact.tile([P, B], dt.float32)
                nc.scalar.activation(g_t[:], pre_cc[:], AF.Tanh)

                pre_fo = gact.tile([P, 2 * B], dt.float32)
                nc.vector.tensor_add(pre_fo[:], ps_fo[:], xt_t[:, 0:2 * B])
                fo = gact.tile([P, 2 * B], dt.float32)
                nc.scalar.activation(fo[:], pre_fo[:], AF.Sigmoid)

                if debug and t == 1:
                    nc.sync.dma_start(dbg_x.ap(), xt_t[:])
                    dpre = gact.tile([P, G * B], dt.float32)
                    nc.vector.tensor_copy(dpre[:, 0:2 * B], pre_fo[:])
                    nc.vector.tensor_copy(dpre[:, ds(2 * B, B)], pre_cc[:])
                    nc.sync.dma_start(dbg_pre.ap(), dpre[:])
                    dh = gact.tile([P, NCORES, B], dt.float32)
                    dcp = nc.vector.tensor_copy(dh[:], hT[:, par])
                    add_dep_helper(dcp.ins, rnop.ins, sync=True,
                                   reason="dbg after receive wait")
                    nc.sync.dma_start(dbg_h.ap(), dh[:])

                c1 = gact.tile([P, B], dt.float32)
                nc.vector.tensor_mul(c1[:], fo[:, 0:B], cT)
                nc.vector.tensor_add(cT, c1[:], g_t[:])
                nc.vector.tensor_mul(hT[:, npar, 0, :], fo[:, ds(B, B)], cT)
                if not no_comm:
                    gnop = nc.gpsimd.nop()
                    add_dep_helper(gnop.ins, prev_trig.ins, sync=False,
                                   reason="preps after previous trigger")
                    gnop._wait_ge(tsem, t + 1)
                    if t >= 4:
                        gnop2 = nc.gpsimd.nop()
                        add_dep_helper(gnop2.ins, gnop.ins, sync=False,
                                       reason="ring reclaim nop chain")
                        post_waits.append((gnop2, lsem[t % 4],
                                           112 * (t // 4)))
                        gnop = gnop2
                    for k in range(1, NCORES):
                        rd = [None] * 8
                        rd[k] = (0, k)
                        pr = nc.gpsimd.remote_dma_broadcast(
                            hT[:, npar, k, :], hT[:, npar, 0, :],
                            rsem[npar], lsem[t % 4], rdests=rd)
                        if k == 1:
                            add_dep_helper(pr.ins, gnop.ins, sync=False,
                                           reason="preps after gate nop")
                    prev_trig = nc.gpsimd.trigger_dma(
                        count=None).then_inc(tsem)

                hf = ysb.tile([P, B], dt.float32)
                nc.vector.tensor_mul(hf[:], fo[:, ds(B, B)], cT)
                yp = yps.tile([B, P], dt.float32)
                nc.tensor.transpose(yp[:], hf[:], ident)
                yo = ysb.tile([B, P], dt.float32)
                nc.scalar.activation(yo[:], yp[:], AF.Copy)
                nc.sync.dma_start(y.ap()[:, t, :], yo[:])
                if t == s_steps - 1:
                    nc.sync.dma_start(hc.ap()[0], yo[:])
                    cp = yps.tile([B, P], dt.float32)
                    nc.tensor.transpose(cp[:], cT, ident)
                    co = ysb.tile([B, P], dt.float32)
                    nc.scalar.activation(co[:], cp[:], AF.Copy)
                    nc.sync.dma_start(hc.ap()[1], co[:])

        # stamp out (late, off the critical path)
        with tc.tile_pool(name="sto", bufs=1) as sto:
            so = sto.tile([P, NCORES, 8], dt.float32)
            cp = nc.vector.tensor_copy(so[:], stp)
            if not no_comm:
                vnop = nc.vector.nop()
                post_waits.append((vnop, ssem, 14))
                add_dep_helper(cp.ins, vnop.ins, sync=False,
                               reason="stamp copy after stamp exchange")
            nc.sync.dma_start(stamp_o.ap(), so[:])

    for ins, sem, val in post_waits:
        ins._wait_ge(sem, val)
    nc.compile()
    return nc


def prep_shared(x, s_steps: int = S):
    # xT[j, p, t*B + b] = x[b, t, j*P + p]
    xt = np.ascontiguousarray(np.transpose(x[:, :s_steps, :], (2, 1, 0)))
    return {"xT": xt.astype(BF16).reshape(KT, P, s_steps * B)}


def prep_core(inputs, c, m_row):
    ws, us, bs = [], [], []
    rows = slice(c * HS, (c + 1) * HS)
    inputs = {k: np.asarray(v, np.float32) for k, v in inputs.items()
              if k != "x"}
    for g in "foc":
        Wg = inputs[f"W_{g}"][rows]          # [128, I]
        ws.append(np.stack([np.ascontiguousarray(Wg[:, j * P:(j + 1) * P].T)
                            for j in range(KT)]))
        Ug = inputs[f"U_{g}"][rows]          # [128, H]
        us.append(np.stack(
            [np.ascontiguousarray(Ug[:, m_row[k] * P:(m_row[k] + 1) * P].T)
             for k in range(NCORES)]))
        bs.append(inputs[f"b_{g}"][rows])
    # [G, KT, P(p), P(q)] -> SBUF layout [P(p), G, KT, P(q)]
    return {
        "wT": np.ascontiguousarray(
            np.transpose(np.stack(ws), (2, 0, 1, 3))).astype(BF16),
        "uT": np.ascontiguousarray(
            np.transpose(np.stack(us), (2, 0, 1, 3))).astype(BF16),
        "bias": np.stack(bs, axis=1).astype(np.float32),
        "idv": np.full((P, 8), float(c), np.float32),
        "ident": np.eye(P, dtype=np.float32),
    }


_cache = {}


def _get_program(s_steps, s_in=None, no_comm=False, no_wait=False):
    key = (s_steps, s_in, no_comm, no_wait)
    if key not in _cache:
        _cache[key] = build_program(s_steps, s_in=s_in, no_comm=no_comm,
                                    no_wait=no_wait)
    return _cache[key]


def run_lstm(inputs, s_steps: int = S, m_map=None, _retried=False):
    m_map = [list(r) for r in (m_map or M_MAP)]
    nc = _get_program(s_steps)
    shared = prep_shared(np.asarray(inputs["x"], np.float32), s_steps)
    in_maps = []
    for c in range(NCORES):
        m = dict(shared)
        m.update(prep_core(inputs, c, m_map[c]))
        in_maps.append(m)
    import time
    try:
        res = bass_utils.run_bass_kernel_spmd(nc, in_maps,
                                              core_ids=list(range(NCORES)))
    except Exception:
        # the axon device occasionally reports unrecoverable after a prior
        # crashed session; it clears after the NRT session cycles
        time.sleep(12)
        res = bass_utils.run_bass_kernel_spmd(nc, in_maps,
                                              core_ids=list(range(NCORES)))
    # validate slot ownership against the observed stamps
    observed = []
    ok = True
    for c in range(NCORES):
        st = res.results[c]["stamp"]
        row = [int(round(float(st[0, k, 0]))) for k in range(NCORES)]
        observed.append(row)
        if row != m_map[c]:
            ok = False
    if not ok:
        if _retried:
            raise RuntimeError(f"slot map unstable: {observed}")
        return run_lstm(inputs, s_steps, m_map=observed, _retried=True)

    yf = np.concatenate([res.results[c]["y"] for c in range(NCORES)], axis=2)
    hf = np.concatenate([res.results[c]["hc"][0] for c in range(NCORES)],
                        axis=1)[None]
    cf = np.concatenate([res.results[c]["hc"][1] for c in range(NCORES)],
                        axis=1)[None]
    return yf, hf, cf


def kernel(**inputs):
    return run_lstm(inputs, S)


def bench(inputs, s_steps: int = S, iters: int = 12, s_in: int = None,
          no_comm: bool = False, no_wait: bool = False):
    """Estimate device time by pipelining `iters` executions of the cached
    PJRT executable and timing the drain. Returns ns per execution."""
    import time
    import jax
    import jax.numpy as jnp
    from jax.sharding import Mesh, PartitionSpec
    from jax.experimental.shard_map import shard_map
    from concourse import bass2jax as b2j

    nc = _get_program(s_steps, s_in, no_comm, no_wait)
    b2j.install_neuronx_cc_hook()
    shared = prep_shared(np.asarray(inputs["x"], np.float32), s_in or s_steps)
    in_maps = []
    for c in range(NCORES):
        m = dict(shared)
        m.update(prep_core(inputs, c, M_MAP[c]))
        in_maps.append(m)

    import concourse.mybir as mybir
    partition_name = (nc.partition_id_tensor.name
                      if nc.partition_id_tensor else None)
    in_names, out_names, out_avals, zero_outs = [], [], [], []
    for alloc in nc.m.functions[0].allocations:
        if not isinstance(alloc, mybir.MemoryLocationSet):
            continue
        name = alloc.memorylocations[0].name
        if alloc.kind == "ExternalInput":
            if name != partition_name:
                in_names.append(name)
        elif alloc.kind == "ExternalOutput":
            shape = tuple(alloc.tensor_shape)
            dtype = mybir.dt.np(alloc.dtype)
            out_names.append(name)
            out_avals.append(jax.core.ShapedArray(shape, dtype))
            zero_outs.append(np.zeros(shape, dtype))
    n_params = len(in_names)
    all_names = list(in_names) + list(out_names)
    if partition_name is not None:
        all_names.append(partition_name)

    def _body(*args):
        operands = list(args)
        if partition_name is not None:
            operands.append(b2j.partition_id_tensor())
        return tuple(b2j._bass_exec_p.bind(
            *operands, out_avals=tuple(out_avals), in_names=tuple(all_names),
            out_names=tuple(out_names), lowering_input_output_aliases=(),
            sim_require_finite=False, sim_require_nnan=False, nc=nc))

    devices = jax.devices()[:NCORES]
    mesh = Mesh(np.asarray(devices), ("core",))
    nin = n_params + len(out_names)
    fn = jax.jit(shard_map(_body, mesh=mesh,
                           in_specs=(PartitionSpec("core"),) * nin,
                           out_specs=(PartitionSpec("core"),) * len(out_names),
                           check_rep=False), keep_unused=True)
    concat_in = [np.concatenate([np.asarray(in_maps[c][n])[None]
                                 for c in range(NCORES)]).reshape(
                     NCORES * np.asarray(in_maps[0][n]).shape[0],
                     *np.asarray(in_maps[0][n]).shape[1:])
                 for n in in_names]
    concat_zo = [np.concatenate([z[None]] * NCORES).reshape(
        NCORES * z.shape[0], *z.shape[1:]) for z in zero_outs]
    args = [jax.device_put(a) for a in concat_in + concat_zo]
    # warmup
    r = fn(*args)
    jax.block_until_ready(r)
    t0 = time.time()
    rs = [fn(*args) for _ in range(iters)]
    jax.block_until_ready(rs)
    t1 = time.time()
    return (t1 - t0) / iters * 1e9


if __name__ == "__main__":
    rng = np.random.default_rng(0)
    stdv = 1.0 / np.sqrt(H)
    demo = {"x": rng.standard_normal((B, S, I), dtype=np.float32)}
    for g in "fioc":
        demo[f"W_{g}"] = rng.uniform(-stdv, stdv, (H, I)).astype(np.float32)
        demo[f"U_{g}"] = rng.uniform(-stdv, stdv, (H, H)).astype(np.float32)
        demo[f"b_{g}"] = rng.uniform(-stdv, stdv, (H,)).astype(np.float32)
    out, h, c = run_lstm(demo, 16)
    print("ran", out.shape, h.shape, c.shape)
